# revision 1
# baseline (speedup 1.0000x reference)
"""RSNN spike kernel for Trainium2, 8 NeuronCores, batch-data-parallel.

Reference semantics (fp32):
    hin_t = x_t @ W1.T + hs_{t-1} @ R
    hm_t  = TAU*hm_{t-1}*(1-hs_{t-1}) + hin_t ;  hs_t = (hm_t >= 1)
    om_t  = TAU*om_{t-1}*(1-os_{t-1}) + hs_t @ Wout.T ;  os_t = (om_t >= 1)
    out[:, t, :] = os_t

Sharding: batch 512 -> 64 per core (8 cores), weights replicated, scan local.
Matmul precision:
  - hs @ R, hs @ Wout.T: native fp32 matmuls.
  - x @ W1.T: fp32r hi/lo splits (x = xh+xl, W1 = wh+wl exact; hh+hl+lh
    products exact into fp32 PSUM; dropped xl*wl term ~1e-8).
Elementwise ops replicate the reference's fp32 rounding sequence.
"""
import sys
for p in ('/opt/trn_rl_repo', '/root/.axon_site'):
    if p not in sys.path:
        sys.path.insert(0, p)
import numpy as np

TAU = 0.6
B, T, NI, NH, NO = 512, 100, 700, 1024, 20
NCORES = 8
BL = B // NCORES            # 64 batch per core
CH = 4                      # timesteps per x/z chunk
NCH = T // CH               # chunks
KI = (NI + 127) // 128      # 6 input k-tiles (last = 60 rows)
KH = NH // 128              # 8 hidden k-tiles / m-tiles
RING = 2 * CH               # hs ring depth (steps)
HB = KH * BL                # 512: cols of one step's state tiles


def _to_r(a):
    from neuron_dtypes import static_cast_fp32_to_fp32r, static_cast_fp32r_to_fp32
    shp = a.shape
    r = static_cast_fp32_to_fp32r(np.ascontiguousarray(a, dtype=np.float32).ravel())
    back = static_cast_fp32r_to_fp32(r)
    return np.asarray(back, dtype=np.float32).reshape(shp)


def _split_r(a):
    h = _to_r(a)
    l = _to_r((a.astype(np.float32) - h).astype(np.float32))
    return h, l


def build_program():
    import concourse.tile as tile
    from concourse import mybir, bacc

    F32 = mybir.dt.float32
    F32R = mybir.dt.float32r
    BF16 = mybir.dt.bfloat16
    Alu = mybir.AluOpType

    nc = bacc.Bacc("TRN2", target_bir_lowering=False, debug=False,
                   num_devices=NCORES)

    xh_e = nc.declare_dram_parameter("xh", [NI, T * BL], F32R, isOutput=False)
    xl_e = nc.declare_dram_parameter("xl", [NI, T * BL], F32R, isOutput=False)
    w1h_e = nc.declare_dram_parameter("w1h", [NI, NH], F32R, isOutput=False)
    w1l_e = nc.declare_dram_parameter("w1l", [NI, NH], F32R, isOutput=False)
    rh_e = nc.declare_dram_parameter("rh", [NH, NH], BF16, isOutput=False)
    rl_e = nc.declare_dram_parameter("rl", [NH, NH], BF16, isOutput=False)
    rl2_e = nc.declare_dram_parameter("rl2", [NH, NH], BF16, isOutput=False)
    wouth_e = nc.declare_dram_parameter("wouth", [NH, NO], BF16, isOutput=False)
    woutl_e = nc.declare_dram_parameter("woutl", [NH, NO], BF16, isOutput=False)
    woutl2_e = nc.declare_dram_parameter("woutl2", [NH, NO], BF16, isOutput=False)
    out_e = nc.declare_dram_parameter("out", [T, NO, BL], F32, isOutput=True)

    with tile.TileContext(nc) as tc:
        with (
            tc.tile_pool(name="wpool", bufs=1) as wpool,
            tc.tile_pool(name="xpool", bufs=2) as xpool,
            tc.tile_pool(name="state", bufs=1) as state,
            tc.tile_pool(name="hx", bufs=2) as hxpool,
            tc.tile_pool(name="ew", bufs=2) as ewpool,
            tc.tile_pool(name="ostg", bufs=2) as ostg,
            tc.tile_pool(name="psx", bufs=3, space="PSUM") as psx,
            tc.tile_pool(name="psr", bufs=2, space="PSUM") as psr,
            tc.tile_pool(name="psz", bufs=2, space="PSUM") as psz,
        ):
            # ---- resident weights
            w1h_sb, w1l_sb, r_sb, wo_sb = [], [], [], []
            for k in range(KI):
                kp = min(128, NI - 128 * k)
                th = wpool.tile([128, NH], F32R, name=f"w1h{k}", tag=f"w1h{k}")
                nc.gpsimd.dma_start(th[0:kp, :], w1h_e[128 * k:128 * k + kp, :])
                w1h_sb.append(th)
                tl_ = wpool.tile([128, NH], F32R, name=f"w1l{k}", tag=f"w1l{k}")
                nc.gpsimd.dma_start(tl_[0:kp, :], w1l_e[128 * k:128 * k + kp, :])
                w1l_sb.append(tl_)
            for k in range(KH):
                th = wpool.tile([128, NH], BF16, name=f"rh{k}", tag=f"rh{k}")
                nc.gpsimd.dma_start(th[:], rh_e[128 * k:128 * (k + 1), :])
                tl_ = wpool.tile([128, NH], BF16, name=f"rl{k}", tag=f"rl{k}")
                nc.gpsimd.dma_start(tl_[:], rl_e[128 * k:128 * (k + 1), :])
                tl2 = wpool.tile([128, NH], BF16, name=f"rl2{k}", tag=f"rl2{k}")
                nc.gpsimd.dma_start(tl2[:], rl2_e[128 * k:128 * (k + 1), :])
                r_sb.append((th, tl_, tl2))
            for k in range(KH):
                th = wpool.tile([128, NO], BF16, name=f"woh{k}", tag=f"woh{k}")
                nc.gpsimd.dma_start(th[:], wouth_e[128 * k:128 * (k + 1), :])
                tl_ = wpool.tile([128, NO], BF16, name=f"wol{k}", tag=f"wol{k}")
                nc.gpsimd.dma_start(tl_[:], woutl_e[128 * k:128 * (k + 1), :])
                tl2 = wpool.tile([128, NO], BF16, name=f"wol2{k}", tag=f"wol2{k}")
                nc.gpsimd.dma_start(tl2[:], woutl2_e[128 * k:128 * (k + 1), :])
                wo_sb.append((th, tl_, tl2))

            # ---- persistent state
            hs_ringb = state.tile([128, RING * HB], BF16, name="hs_ringb", tag="hs_ringb")
            zscr = state.tile([128, HB], F32, name="zscr", tag="zscr")
            nc.vector.memset(zscr[:], 0.0)
            for s in range(RING):
                nc.vector.tensor_scalar(
                    hs_ringb[:, s * HB:(s + 1) * HB], zscr[:], 1e30, None, Alu.is_ge)
            v_st = [state.tile([128, HB], F32, name=f"v{p}", tag=f"v{p}") for p in range(2)]
            nc.vector.memset(v_st[0][:], 0.0)
            vo_st = [state.tile([NO, BL], F32, name=f"vo{p}", tag=f"vo{p}") for p in range(2)]
            nc.vector.memset(vo_st[0][:], 0.0)

            def hs_ktileb(t, k):
                base = (t % RING) * HB + k * BL
                return hs_ringb[:, base:base + BL]

            def load_x_chunk(c):
                t0 = c * CH
                xs = []
                for k in range(KI):
                    kp = min(128, NI - 128 * k)
                    th = xpool.tile([128, CH * BL], F32R, name=f"xh{k}", tag=f"xh{k}")
                    tl_ = xpool.tile([128, CH * BL], F32R, name=f"xl{k}", tag=f"xl{k}")
                    nc.gpsimd.dma_start(
                        th[0:kp, :],
                        xh_e[128 * k:128 * k + kp, t0 * BL:(t0 + CH) * BL])
                    nc.gpsimd.dma_start(
                        tl_[0:kp, :],
                        xl_e[128 * k:128 * k + kp, t0 * BL:(t0 + CH) * BL])
                    xs.append((th, tl_))
                return xs

            def x_matmuls_m(c, xs, hinx, ms):
                for m in ms:
                    px = psx.tile([128, CH * BL], F32, name="px", tag="px")
                    first = True
                    for k in range(KI):
                        kp = min(128, NI - 128 * k)
                        xh_t, xl_t = xs[k]
                        combos = ((w1h_sb[k], xh_t), (w1l_sb[k], xh_t),
                                  (w1h_sb[k], xl_t))
                        for ci, (wt, xt) in enumerate(combos):
                            last = (k == KI - 1) and (ci == 2)
                            nc.tensor.matmul(
                                px[:],
                                lhsT=wt[0:kp, 128 * m:128 * (m + 1)],
                                rhs=xt[0:kp, :],
                                start=first, stop=last)
                            first = False
                    for tl in range(CH):
                        nc.scalar.copy(
                            hinx[tl][:, m * BL:(m + 1) * BL],
                            px[:, tl * BL:(tl + 1) * BL])

            def new_hinx():
                return [hxpool.tile([128, HB], F32, name=f"hinx{tl}", tag=f"hinx{tl}")
                        for tl in range(CH)]

            def step(t, hinx_t):
                pr = psr.tile([128, HB], F32, name="pr", tag="pr")
                for m in range(KH):
                    for k in range(KH):
                        nc.tensor.matmul(
                            pr[:, m * BL:(m + 1) * BL],
                            lhsT=r_sb[k][0][:, 128 * m:128 * (m + 1)],
                            rhs=hs_ktileb((t - 1) % RING, k),
                            start=(k == 0), stop=False)
                    for si in (1, 2):
                        for k in range(KH):
                            nc.tensor.matmul(
                                pr[:, m * BL:(m + 1) * BL],
                                lhsT=r_sb[k][si][:, 128 * m:128 * (m + 1)],
                                rhs=hs_ktileb((t - 1) % RING, k),
                                start=False, stop=(si == 2 and k == KH - 1))
                u1 = ewpool.tile([128, HB], F32, name="u1", tag="u1")
                nc.vector.tensor_add(u1[:], hinx_t[:], pr[:])
                u = ewpool.tile([128, HB], F32, name="u", tag="u")
                nc.vector.tensor_add(u[:], u1[:], v_st[t % 2][:])
                base_b = (t % RING) * HB
                nc.vector.tensor_scalar(
                    hs_ringb[:, base_b:base_b + HB], u[:], 1.0, None, Alu.is_ge)
                st_ = ewpool.tile([128, HB], F32, name="st_", tag="st_")
                nc.vector.tensor_scalar(st_[:], u[:], 1.0, TAU, Alu.is_lt, Alu.mult)
                nc.vector.tensor_mul(v_st[(t + 1) % 2][:], u[:], st_[:])

            def zom_chunk(c):
                t0 = c * CH
                pz = psz.tile([NO, CH * BL], F32, name="pz", tag="pz")
                ring4 = hs_ringb[:].rearrange(
                    "p (s k b) -> p s k b", s=RING, k=KH)
                s0 = t0 % RING
                for k in range(KH):
                    rhs = ring4[:, s0:s0 + CH, k, :]
                    for si in range(3):
                        nc.tensor.matmul(
                            pz[:],
                            lhsT=wo_sb[k][si][:],
                            rhs=rhs,
                            start=(k == 0 and si == 0),
                            stop=(k == KH - 1 and si == 2))
                stg = ostg.tile([NO, CH * BL], F32, name="stg", tag="stg")
                for tl in range(CH):
                    t = t0 + tl
                    uo = ostg.tile([NO, BL], F32, name="uo", tag="uo")
                    nc.vector.tensor_add(
                        uo[:], vo_st[t % 2][:], pz[:, tl * BL:(tl + 1) * BL])
                    nc.vector.tensor_scalar(
                        stg[:, tl * BL:(tl + 1) * BL], uo[:], 1.0, None, Alu.is_ge)
                    so_ = ostg.tile([NO, BL], F32, name="so_", tag="so_")
                    nc.vector.tensor_scalar(so_[:], uo[:], 1.0, TAU,
                                            Alu.is_lt, Alu.mult)
                    nc.vector.tensor_mul(vo_st[(t + 1) % 2][:], uo[:], so_[:])
                for tl in range(CH):
                    nc.gpsimd.dma_start(
                        out_e[t0 + tl, :, :], stg[:, tl * BL:(tl + 1) * BL])

            # ---- main pipeline
            xs = load_x_chunk(0)
            hinx_cur = new_hinx()
            x_matmuls_m(0, xs, hinx_cur, range(KH))
            for c in range(NCH):
                if c + 1 < NCH:
                    xs_next = load_x_chunk(c + 1)
                    hinx_next = new_hinx()
                else:
                    hinx_next = None
                for tl in range(CH):
                    step(c * CH + tl, hinx_cur[tl])
                    if hinx_next is not None:
                        x_matmuls_m(c + 1, xs_next, hinx_next, range(2 * tl, 2 * tl + 2))
                zom_chunk(c)
                hinx_cur = hinx_next
    nc.finalize()
    return nc


_CACHED = {}


def kernel(x, W1, R, Wout):
    from concourse.bass_utils import run_bass_kernel_spmd

    x = np.ascontiguousarray(np.asarray(x, dtype=np.float32))
    W1 = np.asarray(W1, dtype=np.float32)
    R = np.asarray(R, dtype=np.float32)
    Wout = np.asarray(Wout, dtype=np.float32)

    xh, xl = _split_r(x)                             # [B, T, NI]
    w1h, w1l = _split_r(np.ascontiguousarray(W1.T))  # [NI, NH]
    import ml_dtypes
    r32 = np.ascontiguousarray(R)
    rh = r32.astype(ml_dtypes.bfloat16)              # bf16 3-term cascade
    res1 = (r32 - rh.astype(np.float32)).astype(np.float32)
    rl = res1.astype(ml_dtypes.bfloat16)
    rl2 = (res1 - rl.astype(np.float32)).astype(np.float32).astype(ml_dtypes.bfloat16)
    wo32 = np.ascontiguousarray(Wout.T)
    wouth = wo32.astype(ml_dtypes.bfloat16)
    wres = (wo32 - wouth.astype(np.float32)).astype(np.float32)
    woutl = wres.astype(ml_dtypes.bfloat16)
    woutl2 = (wres - woutl.astype(np.float32)).astype(np.float32).astype(ml_dtypes.bfloat16)

    if "nc" not in _CACHED:
        _CACHED["nc"] = build_program()
    nc = _CACHED["nc"]
    in_maps = []
    for c in range(NCORES):
        bsl = slice(c * BL, (c + 1) * BL)
        in_maps.append({
            "xh": np.ascontiguousarray(
                xh[bsl].transpose(2, 1, 0)).reshape(NI, T * BL),
            "xl": np.ascontiguousarray(
                xl[bsl].transpose(2, 1, 0)).reshape(NI, T * BL),
            "w1h": w1h, "w1l": w1l, "rh": rh, "rl": rl, "rl2": rl2,
            "wouth": wouth, "woutl": woutl, "woutl2": woutl2,
        })
    res = run_bass_kernel_spmd(nc, in_maps, list(range(NCORES)))
    out = np.empty((B, T, NO), np.float32)
    for c in range(NCORES):
        o = res.results[c]["out"]                    # [T, NO, BL]
        out[c * BL:(c + 1) * BL] = o.transpose(2, 0, 1)
    return out

